# revision 1
# baseline (speedup 1.0000x reference)
"""CrossBatchAttention kernel for 8 Trainium2 NeuronCores.

Strategy: tensor-parallel over heads. 16 heads / 8 cores = 2 heads per core.
Each core computes:
  - qT/kT = (W_slice @ hidden.T)  in [e, b] layout (e = local head*128 + d)
  - v     = (hidden @ Wv_slice.T) in [b, e] layout
  - per head: scoresT[k, q] = kT.T-tiles @ qT (PE-native), exp on ACT,
    diagonal zeroed multiplicatively, row-sums via ones-matmul partition
    reduction, 1/sum broadcast back to [128, q] via a second ones-matmul,
    ctxT[d, q] = v-tiles @ expT accumulated in PSUM, normalized on eviction
  - cross_partial[b, o] = ctxT-tiles @ WoT_slice (accumulate 2 head tiles)
Host: sums the 8 partial cross projections, out = hidden + sigmoid(s)*cross.

All matmul inputs bf16 (fp32 PSUM accumulation). The residual path keeps
hidden in fp32 exactly, and cross contributes only ~2% of output magnitude,
so bf16 compute error is negligible end-to-end.
"""

import numpy as np
import ml_dtypes

B = 2048
H = 2048
NH = 16
HD = 128
NCORES = 8
HL = NH // NCORES          # heads per core = 2
E = HL * HD                # local projection width = 256
P = 128
KT = H // P                # 16 contraction tiles over hidden dim
BT = B // P                # 16 row tiles
NQ = B // 512              # 4 query chunks of 512

_BF16 = ml_dtypes.bfloat16

_compiled = None


def _build():
    import concourse.bass as bass  # noqa: F401
    import concourse.tile as tile
    from concourse import bacc, mybir

    bf = mybir.dt.bfloat16
    f32 = mybir.dt.float32
    Exp = mybir.ActivationFunctionType.Exp
    mult = mybir.AluOpType.mult

    nc = bacc.Bacc(
        "TRN2",
        target_bir_lowering=False,
        debug=False,
        enable_asserts=False,
        num_devices=NCORES,
    )

    hT_d = nc.dram_tensor("hT", [H, B], bf, kind="ExternalInput").ap()
    wqT_d = nc.dram_tensor("wqT", [H, E], bf, kind="ExternalInput").ap()
    wkT_d = nc.dram_tensor("wkT", [H, E], bf, kind="ExternalInput").ap()
    wvT_d = nc.dram_tensor("wvT", [H, E], bf, kind="ExternalInput").ap()
    woT_d = nc.dram_tensor("woT", [E, H], bf, kind="ExternalInput").ap()
    antiI_d = nc.dram_tensor("antiI", [P, P], bf, kind="ExternalInput").ap()
    out_d = nc.dram_tensor("out", [B, H], bf, kind="ExternalOutput").ap()

    with tile.TileContext(nc) as tc:
        with (
            tc.tile_pool(name="const", bufs=1) as constp,
            tc.tile_pool(name="work", bufs=1) as workp,
            tc.tile_pool(name="stream", bufs=4) as streamp,
            tc.tile_pool(name="psA", bufs=2, space="PSUM") as psA,
            tc.tile_pool(name="psB", bufs=1, space="PSUM") as psB,
        ):
            # ---------------- input DMA ----------------
            hT_sb = constp.tile([P, KT * B], bf)
            for kt in range(KT):
                nc.sync.dma_start(hT_sb[:, kt * B:(kt + 1) * B], hT_d[kt * P:(kt + 1) * P, :])
            wq_sb = constp.tile([P, KT * E], bf)
            wk_sb = constp.tile([P, KT * E], bf)
            wv_sb = constp.tile([P, KT * E], bf)
            for w_sb, w_d in ((wq_sb, wqT_d), (wk_sb, wkT_d), (wv_sb, wvT_d)):
                for kt in range(KT):
                    nc.sync.dma_start(w_sb[:, kt * E:(kt + 1) * E], w_d[kt * P:(kt + 1) * P, :])
            wo_sb = constp.tile([P, HL * H], bf)
            for et in range(HL):
                nc.sync.dma_start(wo_sb[:, et * H:(et + 1) * H], woT_d[et * P:(et + 1) * P, :])
            antiI = constp.tile([P, P], bf)
            nc.sync.dma_start(antiI[:], antiI_d[:])
            ones_k = constp.tile([P, 1], bf)
            nc.gpsimd.memset(ones_k[:], 1.0)
            ones_m = constp.tile([1, P], f32)
            nc.gpsimd.memset(ones_m[:], 1.0)
            zbias = constp.tile([P, 1], f32)
            nc.gpsimd.memset(zbias[:], 0.0)

            qT_sb = workp.tile([P, HL * B], bf)   # [d, b] per head at h*B
            kT_sb = workp.tile([P, HL * B], bf)
            v_sb = workp.tile([P, BT * E], bf)    # [b%128, bt*E + e]
            ctxT_sb = workp.tile([P, HL * B], bf)

            # ---------------- q/k projections (out = [e, b]) ----------------
            for dst, w_sb in ((qT_sb, wq_sb), (kT_sb, wk_sb)):
                for et in range(HL):
                    for bh in range(B // 1024):
                        psp = psA.tile([P, 1024], f32, tag="A")
                        for kt in range(KT):
                            for b2 in range(2):
                                bc = bh * 2 + b2
                                nc.tensor.matmul(
                                    psp[:, b2 * 512:(b2 + 1) * 512],
                                    lhsT=w_sb[:, kt * E + et * P: kt * E + (et + 1) * P],
                                    rhs=hT_sb[:, kt * B + bc * 512: kt * B + (bc + 1) * 512],
                                    start=(kt == 0),
                                    stop=(kt == KT - 1),
                                )
                        nc.any.tensor_copy(dst[:, et * B + bh * 1024: et * B + (bh + 1) * 1024], psp[:])

            # ---------------- v projection (out = [b, e]) ----------------
            for bt in range(BT):
                psv = psA.tile([P, E], f32, tag="A")
                for kt in range(KT):
                    nc.tensor.matmul(
                        psv[:],
                        lhsT=hT_sb[:, kt * B + bt * P: kt * B + (bt + 1) * P],
                        rhs=wv_sb[:, kt * E:(kt + 1) * E],
                        start=(kt == 0),
                        stop=(kt == KT - 1),
                    )
                nc.any.tensor_copy(v_sb[:, bt * E:(bt + 1) * E], psv[:])

            # ---------------- attention per local head ----------------
            for h in range(HL):
                ctxps = psB.tile([P, B], f32, tag="B")
                acc = workp.tile([P, B], bf, tag="acc", bufs=2)
                for kt in range(KT):
                    for qh in range(2):
                        pss = psA.tile([P, 1024], f32, tag="A")
                        for q2 in range(2):
                            qc = qh * 2 + q2
                            nc.tensor.matmul(
                                pss[:, q2 * 512:(q2 + 1) * 512],
                                lhsT=kT_sb[:, h * B + kt * P: h * B + (kt + 1) * P],
                                rhs=qT_sb[:, h * B + qc * 512: h * B + (qc + 1) * 512],
                                start=True,
                                stop=True,
                            )
                        ex = streamp.tile([P, 1024], bf, tag="exp")
                        nc.scalar.activation(ex[:], pss[:], Exp, bias=zbias[:, 0:1])
                        if kt // 8 == qh:
                            off = kt * P - qh * 1024
                            nc.vector.tensor_tensor(
                                ex[:, off:off + P], ex[:, off:off + P], antiI[:], op=mult
                            )
                        if kt == 0:
                            nc.vector.tensor_copy(acc[:, qh * 1024:(qh + 1) * 1024], ex[:])
                        else:
                            nc.vector.tensor_add(
                                acc[:, qh * 1024:(qh + 1) * 1024],
                                acc[:, qh * 1024:(qh + 1) * 1024],
                                ex[:],
                            )
                        for q2 in range(2):
                            qc = qh * 2 + q2
                            nc.tensor.matmul(
                                ctxps[:, qc * 512:(qc + 1) * 512],
                                lhsT=v_sb[:, kt * E + h * P: kt * E + h * P + P],
                                rhs=ex[:, q2 * 512:(q2 + 1) * 512],
                                start=(kt == 0),
                                stop=(kt == KT - 1),
                            )
                # normalization: sums over k via ones-matmul, reciprocal,
                # broadcast along partitions via second ones-matmul
                for qc in range(NQ):
                    pssum = psA.tile([1, 512], f32, tag="A")
                    nc.tensor.matmul(
                        pssum[:], lhsT=ones_k[:], rhs=acc[:, qc * 512:(qc + 1) * 512],
                        start=True, stop=True,
                    )
                    rrow = workp.tile([1, 512], f32, tag="rrow", bufs=2)
                    nc.vector.reciprocal(rrow[:], pssum[:])
                    psrb = psA.tile([P, 512], f32, tag="A")
                    nc.tensor.matmul(psrb[:], lhsT=ones_m[:], rhs=rrow[:], start=True, stop=True)
                    rb = workp.tile([P, 512], f32, tag="rb", bufs=2)
                    nc.scalar.copy(rb[:], psrb[:])
                    nc.vector.tensor_tensor(
                        ctxT_sb[:, h * B + qc * 512: h * B + (qc + 1) * 512],
                        ctxps[:, qc * 512:(qc + 1) * 512],
                        rb[:],
                        op=mult,
                    )

            # ---------------- output projection (partial) ----------------
            for bt in range(BT):
                for oh in range(2):
                    psx = psA.tile([P, 1024], f32, tag="A")
                    for et in range(HL):
                        for o2 in range(2):
                            nc.tensor.matmul(
                                psx[:, o2 * 512:(o2 + 1) * 512],
                                lhsT=ctxT_sb[:, et * B + bt * P: et * B + (bt + 1) * P],
                                rhs=wo_sb[:, et * H + oh * 1024 + o2 * 512: et * H + oh * 1024 + (o2 + 1) * 512],
                                start=(et == 0),
                                stop=(et == HL - 1),
                            )
                    xo = streamp.tile([P, 1024], bf, tag="xo")
                    nc.any.tensor_copy(xo[:], psx[:])
                    nc.sync.dma_start(
                        out_d[bt * P:(bt + 1) * P, oh * 1024:(oh + 1) * 1024], xo[:]
                    )

    nc.compile()
    return nc


def _get_compiled():
    global _compiled
    if _compiled is None:
        _compiled = _build()
    return _compiled


def _numpy_reference(hidden_states, attention_mask, Wq, Wk, Wv, Wo, scale_param):
    hs = np.asarray(hidden_states, np.float64)
    q = (hs @ np.asarray(Wq, np.float64).T).reshape(B, NH, HD).transpose(1, 0, 2)
    k = (hs @ np.asarray(Wk, np.float64).T).reshape(B, NH, HD).transpose(1, 0, 2)
    v = (hs @ np.asarray(Wv, np.float64).T).reshape(B, NH, HD).transpose(1, 0, 2)
    scores = np.einsum("hqd,hkd->hqk", q, k) / (HD ** 0.5)
    eye = np.eye(B, dtype=bool)
    scores = np.where(eye[None, :, :], -np.inf, scores)
    mask = np.asarray(attention_mask, bool)
    scores = np.where(mask[None, None, :], scores, -np.inf)
    m = scores.max(axis=-1, keepdims=True)
    m = np.where(np.isfinite(m), m, 0.0)
    e = np.exp(scores - m)
    s = e.sum(axis=-1, keepdims=True)
    attn = np.where(s > 0, e / np.maximum(s, 1e-300), 0.0)
    ctx = np.einsum("hqk,hkd->hqd", attn, v)
    ctx = ctx.transpose(1, 0, 2).reshape(B, H)
    cross = ctx @ np.asarray(Wo, np.float64).T
    scale = 1.0 / (1.0 + np.exp(-float(np.asarray(scale_param).reshape(-1)[0])))
    return (hs + scale * cross).astype(np.float32)


LAST_RESULTS = None


def kernel(hidden_states, attention_mask, Wq, Wk, Wv, Wo, scale_param):
    hs = np.asarray(hidden_states, np.float32)
    mask = np.asarray(attention_mask, bool)
    if not mask.all():
        return _numpy_reference(hidden_states, mask, Wq, Wk, Wv, Wo, scale_param)

    from concourse import bass_utils

    nc = _get_compiled()

    hT = np.ascontiguousarray(hs.T).astype(_BF16)
    antiI = (1.0 - np.eye(P, dtype=np.float32)).astype(_BF16)
    Wq = np.asarray(Wq, np.float32)
    Wk = np.asarray(Wk, np.float32)
    Wv = np.asarray(Wv, np.float32)
    Wo = np.asarray(Wo, np.float32)

    in_maps = []
    for c in range(NCORES):
        rs = slice(c * E, (c + 1) * E)
        in_maps.append({
            "hT": hT,
            "wqT": np.ascontiguousarray(Wq[rs, :].T / np.float32(HD ** 0.5)).astype(_BF16),
            "wkT": np.ascontiguousarray(Wk[rs, :].T).astype(_BF16),
            "wvT": np.ascontiguousarray(Wv[rs, :].T).astype(_BF16),
            "woT": np.ascontiguousarray(Wo[:, rs].T).astype(_BF16),
            "antiI": antiI,
        })

    import os
    res = bass_utils.run_bass_kernel_spmd(
        nc, in_maps, core_ids=list(range(NCORES)),
        trace=bool(os.environ.get("KERNEL_TRACE")),
    )
    global LAST_RESULTS
    LAST_RESULTS = res

    cross = np.zeros((B, H), np.float32)
    for r in res.results:
        cross += np.asarray(r["out"], np.float32)
    scale = np.float32(1.0 / (1.0 + np.exp(-float(np.asarray(scale_param).reshape(-1)[0]))))
    return (hs + scale * cross).astype(np.float32)
